# revision 31
# baseline (speedup 1.0000x reference)
"""Causal self-attention Trainium2 kernel (8 NeuronCores, SPMD).

Sharding: 8 cores = 2 batches x 4 head-groups (4 heads of 64 dims each).
Each core computes full-sequence attention for its 4 heads plus the
partial output projection for its 256 y-columns; the host sums the 4
partials per batch and adds the (bv @ Wp + bp) bias row — the v-bias
folds through softmax exactly since attention rows sum to 1.

Layout strategy (no on-device transposes anywhere):
  - host supplies x[b].T t-chunk-major as xT [4, C, 512] (bf16) and
    wq/wk column-halved as [2, C, 128]: every input DMA moves fully
    contiguous >=2KB-per-partition descriptor lines (strided descriptors
    measured ~50GB/s vs ~95GB/s/queue contiguous); chunk tb=0 is split
    3-ways across the sync/scalar/gpsimd queues so the first projection
    starts ~13us in, with early dummy PE matmuls holding the p-state up
  - qT, kT produced in [m, t] layout (W stationary, xT moving); the
    PSUM->SBUF copy runs on the then-idle scalar engine as an Identity
    activation with the per-partition bias AP (exp/identity/copy share
    one activation table, so no ACT_TABLE_LOAD thrash)
  - v produced in natural [t, m] layout, augmented with a ones column
    per head (M=65) so the attention-value matmul also emits the
    softmax denominator row for free
  - attT[j, i] = sum_d kT[d,j] qT[d,i]  (kT stationary K=64; two heads
    run concurrently via row-tiled base partitions 0/64)
  - exp on ScalarE (fused 1/sqrt(64) scale) in 1024-wide chunks;
    diagonal 128-tiles masked with a multiplicative mask tile on DVE
  - PT rows stored triangularly (only i >= 128*jt) — fits in SBUF
    alongside xT, letting the second head-pair's q/k projections and
    the deferred v tiles interleave into pair 0's scalar-bound stretch
  - 1/s via DVE reciprocal_approx_fast, broadcast across partitions on
    the otherwise-idle GpSimd engine (partition_broadcast) — keeps the
    PE free of K=1 broadcast matmuls; pair-0's scale-mults all drain at
    pair-1's prologue so only pair-1's two remain on the tail
  - pair-1 pipeline: scale-mults for i-block N run while the
    attention-value pass for N+1 runs on the PE; out-proj follows
  - out[t, n] partials bf16, DMA'd per 512-column half on whichever
    queues are idle at that stage (gpsimd mid-run, scalar after exp
    ends, 3-way fan-out for the last block) to shrink the drain tail
  - pair-0's first score rows + exps are emitted inside the projection
    tail so the ~83us scalar exp stream (the saturated engine through
    the middle) starts ~25us in; late phase-1 copies route to DVE so
    they never extend the scalar stream
  - power note: HAM clock-gates the PE (k=8 -> k=4) under sustained
    dense multi-engine activity; a FULLY-interleaved schedule measured
    slower purely from added throttle — this lite interleave is the
    measured sweet spot
"""

import sys

for _p in ("/opt/trn_rl_repo",):
    if _p not in sys.path:
        sys.path.insert(0, _p)

from contextlib import ExitStack

import ml_dtypes
import numpy as np

import concourse.bass as bass
import concourse.tile as tile
from concourse import bacc, mybir
from concourse.bass_utils import run_bass_kernel_spmd

BF16 = mybir.dt.bfloat16
F32 = mybir.dt.float32
NP_BF16 = ml_dtypes.bfloat16

PHASE_MARKS = []


def _mark(nc, label):
    nm = nc.get_next_instruction_name()  # burn one name as a phase boundary
    PHASE_MARKS.append((label, int(nm.split("-")[1])))


B, T, C = 2, 2048, 1024
H, D = 16, 64
N_CORES = 8
GROUPS = 4          # head groups (cores per batch)
MH = C // GROUPS    # 256 columns per core (4 heads)
LH = MH // D        # 4 local heads
CT = C // 128       # 8 contraction tiles
TT = T // 128       # 16 sequence tiles of 128
IB = T // 512       # 4 i-blocks of 512
SCALE = 1.0 / np.sqrt(D)
N_WARM = 16         # dummy matmuls covering the initial DMA wait

# triangular PT layout: row jt holds cols i in [128*jt, T)
TRI_OFF = [0] * TT
for _jt in range(1, TT):
    TRI_OFF[_jt] = TRI_OFF[_jt - 1] + (T - 128 * (_jt - 1))
TRI = TRI_OFF[-1] + (T - 128 * (TT - 1))   # 17408


def _causal_mask() -> np.ndarray:
    """mask[j, i] = 1.0 if j <= i else 0 (bf16), [128, 128]."""
    j = np.arange(128)[:, None]
    i = np.arange(128)[None, :]
    return (j <= i).astype(NP_BF16)


def emit_kernel(nc, xT_d, wq_d, wk_d, wv_d, wp_d, bq_d, bk_d, out_d, mask_d):
    with tile.TileContext(nc) as tc, ExitStack() as ctx:
        _mark(nc, "setup")
        # ---- long-lived tiles -------------------------------------------
        keep = ctx.enter_context(tc.tile_pool(name="keep", bufs=1))
        qT_s = keep.tile([128, 2, T], BF16, tag="qT")
        kT_s = keep.tile([128, 2, T], BF16, tag="kT")
        v_s = keep.tile([128, TT, LH, D + 1], BF16, tag="v")
        yTn_s = keep.tile([128, 2, T], BF16, tag="yTn")
        wp_s = keep.tile([128, 2, C], BF16, tag="wp")
        mask_st = keep.tile([128, 128], BF16, tag="mask_st")
        mask_s = keep.tile([128, 128], BF16, tag="mask")
        bq_st = keep.tile([128, 2], F32, tag="bq_st")
        bq_s = keep.tile([128, 2], F32, tag="bq")
        bk_st = keep.tile([128, 2], F32, tag="bk_st")
        bk_s = keep.tile([128, 2], F32, tag="bk")
        rs_all = keep.tile([1, 2, IB, 2, 512], BF16, tag="rs_all")  # 1/s rows
        warm_s = keep.tile([128, 256], BF16, tag="warm")

        nc.vector.memset(warm_s[:], 1.0)

        # PE warm-up first thing: dummy matmuls keep the p-state ramping
        # while the input DMAs land (PE queue is in-order, so these retire
        # before the first projection without delaying it)
        with tc.tile_pool(name="warm_ps", bufs=1, space="PSUM") as wpool:
            for _ in range(N_WARM):
                wps_t = wpool.tile([128, 512], F32, tag="warm_ps")
                nc.tensor.matmul(
                    wps_t[:, 0:256], warm_s[:, 0:128], warm_s[:], start=True, stop=True
                )

        # projection inputs stay alive through pair 0's attention so the
        # mt=1 q/k projections can interleave there
        pin = ctx.enter_context(tc.tile_pool(name="proj_in", bufs=1))
        xT_s = pin.tile([128, IB, CT, 512], BF16, tag="xT")  # t-chunk major
        wq_s = pin.tile([128, CT, MH], BF16, tag="wq")
        wk_s = pin.tile([128, CT, MH], BF16, tag="wk")
        wv_s = pin.tile([128, CT, MH], BF16, tag="wv")
        xT_r = xT_d.ap().rearrange("c (p o) t -> c p o t", p=128)
        wq_r = wq_d.ap().rearrange("h (p o) m -> h p o m", p=128)
        wk_r = wk_d.ap().rearrange("h (p o) m -> h p o m", p=128)
        wv_r = wv_d.ap().rearrange("(p o) m -> p o m", p=128)

        # consts first (tiny, land immediately), staged through a DVE copy:
        # consumers then depend on DVE program order instead of a DMA
        # semaphore (walrus 1-wait limit)
        # xT arrives t-chunk-major ([4, C, 512] in DRAM, chunk-major SBUF
        # tile): fully-contiguous 4KB-per-partition descriptor lines. The
        # first chunk (tb=0) is split three ways across all hwdge queues so
        # the first projection group can start ~13us in; later chunks
        # alternate sync/scalar halves.
        nc.sync.dma_start(wq_s[:, :, 0:128], wq_r[0])
        nc.scalar.dma_start(xT_s[:, 0, 0:3, :], xT_r[0, :, 0:3, :])
        nc.gpsimd.dma_start(xT_s[:, 0, 3:6, :], xT_r[0, :, 3:6, :])
        nc.sync.dma_start(xT_s[:, 0, 6:CT, :], xT_r[0, :, 6:CT, :])
        nc.gpsimd.dma_start(wk_s[:, :, 0:128], wk_r[0])
        nc.gpsimd.dma_start(mask_st[:], mask_d.ap())
        nc.gpsimd.dma_start(bq_st[:], bq_d.ap().rearrange("(o p) -> p o", p=128))
        nc.gpsimd.dma_start(bk_st[:], bk_d.ap().rearrange("(o p) -> p o", p=128))
        for _tb in range(1, IB):
            nc.sync.dma_start(xT_s[:, _tb, 0:4, :], xT_r[_tb, :, 0:4, :])
            nc.scalar.dma_start(xT_s[:, _tb, 4:CT, :], xT_r[_tb, :, 4:CT, :])
            if _tb == 1:
                nc.gpsimd.dma_start(wv_s[:], wv_r[:])
        nc.gpsimd.dma_start(wq_s[:, :, 128:MH], wq_r[1])
        nc.gpsimd.dma_start(wk_s[:, :, 128:MH], wk_r[1])
        wp_r = wp_d.ap().rearrange("(o p) n -> p o n", p=128)
        nc.gpsimd.dma_start(wp_s[:], wp_r[:])
        nc.vector.tensor_copy(mask_s[:], mask_st[:])
        nc.vector.tensor_copy(bq_s[:], bq_st[:])
        nc.vector.tensor_copy(bk_s[:], bk_st[:])
        nc.vector.memset(v_s[:, :, :, D : D + 1], 1.0)

        def proj_group(ps, w_s, b_s, dst, mt, tb, on_scalar=True):
            """one [128, 512] column block of qT or kT (8-deep K accum)."""
            for ct in range(CT):
                nc.tensor.matmul(
                    ps[:, 0:512],
                    w_s[:, ct, mt * 128 : (mt + 1) * 128],
                    xT_s[:, tb, ct, :],
                    start=(ct == 0),
                    stop=(ct == CT - 1),
                )
            if on_scalar:
                nc.scalar.activation(
                    dst[:, mt, tb * 512 : (tb + 1) * 512],
                    ps[:, 0:512],
                    mybir.ActivationFunctionType.Identity,
                    bias=b_s[:, mt : mt + 1],
                )
            else:
                nc.vector.tensor_scalar(
                    dst[:, mt, tb * 512 : (tb + 1) * 512],
                    ps[:, 0:512],
                    b_s[:, mt : mt + 1],
                    None,
                    mybir.AluOpType.add,
                )

        def v_group(tt, ps, on_scalar=True):
            # v natural [t, m]  (xT stationary)
            for ct in range(CT):
                nc.tensor.matmul(
                    ps[:, 0:MH],
                    xT_s[:, tt // 4, ct, (tt % 4) * 128 : (tt % 4) * 128 + 128],
                    wv_s[:, ct, :],
                    start=(ct == 0),
                    stop=(ct == CT - 1),
                )
            if on_scalar:
                nc.scalar.activation(
                    v_s[:, tt, :, 0:D],
                    ps[:, 0:MH].rearrange("p (h d) -> p h d", h=LH),
                    mybir.ActivationFunctionType.Copy,
                )
            else:
                nc.vector.tensor_copy(
                    v_s[:, tt, :, 0:D],
                    ps[:, 0:MH].rearrange("p (h d) -> p h d", h=LH),
                )

        # ---- attention with early-start exp -----------------------------
        # Scalar (exp) is the saturated engine through the middle of the
        # run, so the first score chunks + exps are emitted inside the
        # projection tail: the 83us exp stream starts ~17us earlier.
        # PSUM: att(4) + yt(2) + proj(2) banks during pair 0; the proj
        # pool swaps out for the out-proj pool at pair 1.
        with (
            tc.tile_pool(name="pt", bufs=1) as ptp,
            tc.tile_pool(name="att_ps", bufs=2, space="PSUM") as aps,
            tc.tile_pool(name="norm", bufs=2) as npool,
            tc.tile_pool(name="out_sb", bufs=4) as osb,
            tc.tile_pool(name="yt_ps", bufs=2, space="PSUM") as yps,
        ):
            out_r = out_d.ap().rearrange("(tt p) n -> tt p n", p=128)
            yTu_by_p = [None, None]
            pps_cm = tc.tile_pool(name="proj_ps", bufs=2, space="PSUM")
            pps = pps_cm.__enter__()
            fill_pool = [(pps, "proj_ps")]
            ops_box = [None]

            def v_group(tt, ps, on_scalar=False):
                # v natural [t, m]  (xT stationary)
                for ct in range(CT):
                    nc.tensor.matmul(
                        ps[:, 0:MH],
                        xT_s[:, tt // 4, ct, (tt % 4) * 128 : (tt % 4) * 128 + 128],
                        wv_s[:, ct, :],
                        start=(ct == 0),
                        stop=(ct == CT - 1),
                    )
                if on_scalar:
                    nc.scalar.activation(
                        v_s[:, tt, :, 0:D],
                        ps[:, 0:MH].rearrange("p (h d) -> p h d", h=LH),
                        mybir.ActivationFunctionType.Copy,
                    )
                else:
                    nc.vector.tensor_copy(
                        v_s[:, tt, :, 0:D],
                        ps[:, 0:MH].rearrange("p (h d) -> p h d", h=LH),
                    )

            def attT_chunk(p, PT, jt, off):
                """attT + exp (+ diag mask) for row jt, cols off..off+1024."""
                ia = 128 * jt
                base = TRI_OFF[jt]
                cw = min(1024, T - ia - off)
                for lh in range(2):
                    att_ps = aps.tile([128, 1024], F32, tag="att_ps")
                    prow = slice(64 * lh, 64 * lh + 64)
                    for s5 in range(0, cw, 512):
                        nn = min(512, cw - s5)
                        nc.tensor.matmul(
                            att_ps[:, s5 : s5 + nn],
                            kT_s[prow, p, jt * 128 : (jt + 1) * 128],
                            qT_s[prow, p, ia + off + s5 : ia + off + s5 + nn],
                            start=True,
                            stop=True,
                        )
                    nc.scalar.activation(
                        PT[lh][:, base + off : base + off + cw],
                        att_ps[:, :cw],
                        mybir.ActivationFunctionType.Exp,
                        scale=float(SCALE),
                    )
                    if off == 0:
                        # diagonal 128x128 tile: zero j > i
                        nc.vector.tensor_tensor(
                            PT[lh][:, base : base + 128],
                            PT[lh][:, base : base + 128],
                            mask_s[:],
                            mybir.AluOpType.mult,
                        )

            def attT_rows(p, PT, jts, off0=0):
                """chunk-major over a group of rows: all chunk0 first."""
                offs = [off0] * len(jts)
                while True:
                    emitted_any = False
                    for k, jt in enumerate(jts):
                        if offs[k] < T - 128 * jt:
                            attT_chunk(p, PT, jt, offs[k])
                            offs[k] += 1024
                            emitted_any = True
                    if not emitted_any:
                        break

            def new_PT():
                return [
                    ptp.tile([128, TRI], BF16, tag=f"PT{lh}", name=f"PT{lh}")
                    for lh in range(2)
                ]

            # ---- phase 1: mt=0 projections + v, pair-0 scores start -----
            PT0 = new_PT()
            _mark(nc, "proj_qkv")
            for tb in range(IB):
                late = tb >= 2   # scalar is exp-busy from here on
                for w_s, b_s, dst in ((wq_s, bq_s, qT_s), (wk_s, bk_s, kT_s)):
                    ps = pps.tile([128, 512], F32, tag="proj_ps", name="proj_ps")
                    proj_group(ps, w_s, b_s, dst, 0, tb, on_scalar=not late)
                if tb == 1:
                    for tt in (0, 1):
                        ps = pps.tile([128, 512], F32, tag="proj_ps", name="v_ps")
                        v_group(tt, ps, on_scalar=True)
                elif tb == 2:
                    attT_chunk(0, PT0, 0, 0)
                    attT_chunk(0, PT0, 1, 0)
                    for tt in (2, 3):
                        ps = pps.tile([128, 512], F32, tag="proj_ps", name="v_ps")
                        v_group(tt, ps)
                elif tb == 3:
                    attT_chunk(0, PT0, 2, 0)
                    attT_chunk(0, PT0, 3, 0)
                    for tt in (4, 5, 6, 7):
                        ps = pps.tile([128, 512], F32, tag="proj_ps", name="v_ps")
                        v_group(tt, ps)

            # PE filler work for the scalar-bound attention stretches:
            # mt=1 q/k projection groups and the deferred v tiles 8-15
            def _mt1(w_s, b_s, dst, tb):
                def emit():
                    pool, tag = fill_pool[0]
                    ps = pool.tile([128, 512], F32, tag=tag, name="fill_ps")
                    proj_group(ps, w_s, b_s, dst, 1, tb, on_scalar=False)
                return emit

            def _vg(tt):
                def emit():
                    pool, tag = fill_pool[0]
                    ps = pool.tile([128, 512], F32, tag=tag, name="fill_ps")
                    v_group(tt, ps)
                return emit

            fill_sched = {
                (0, 0): [_mt1(wq_s, bq_s, qT_s, 0), _mt1(wq_s, bq_s, qT_s, 1),
                         _vg(8), _vg(9)],
                (0, 1): [_mt1(wq_s, bq_s, qT_s, 2), _mt1(wq_s, bq_s, qT_s, 3),
                         _vg(10), _vg(11)],
                (0, 2): [_mt1(wk_s, bk_s, kT_s, 0), _vg(12), _vg(13)],
                (0, 3): [_mt1(wk_s, bk_s, kT_s, 1), _vg(14), _vg(15)],
                (1, -1): [_mt1(wk_s, bk_s, kT_s, 2)],
                (1, 0): [_mt1(wk_s, bk_s, kT_s, 3)],
            }

            def fill(key):
                for emit in fill_sched.pop(key, []):
                    emit()

            for p in range(2):
                PT = PT0 if p == 0 else new_PT()
                yTu = npool.tile([64, 8, 512], BF16, tag="yTu", name="yTu")
                yTu_by_p[p] = yTu

                def av_block(ib):
                    """attention @ v for i-block ib; returns yT_ps pair."""
                    yT_ps = [
                        yps.tile([D + 1, 512], F32, tag="yT_ps", name=f"yT_ps{lh}")
                        for lh in range(2)
                    ]
                    for jt in range(4 * ib + 4):
                        for lh in range(2):
                            ia = 128 * jt
                            c0 = max(512 * ib, ia)
                            nc.tensor.matmul(
                                yT_ps[lh][:, c0 - 512 * ib : 512],
                                v_s[:, jt, 2 * p + lh, :],
                                PT[lh][
                                    :,
                                    TRI_OFF[jt]
                                    + c0
                                    - ia : TRI_OFF[jt]
                                    + 512 * ib
                                    + 512
                                    - ia,
                                ],
                                start=(jt == 0),
                                stop=(jt == 4 * ib + 3),
                            )
                    return yT_ps

                def stash_recip(ib, yT_ps):
                    """denominators + 1/s first (they gate s_mults), then y."""
                    st = npool.tile([1, 1024], F32, tag="st", name="st", bufs=1)
                    for lh in range(2):
                        nc.vector.tensor_copy(
                            st[0:1, lh * 512 : (lh + 1) * 512], yT_ps[lh][D : D + 1, :]
                        )
                    rf = npool.tile([1, 1024], F32, tag="rf", name="rf", bufs=1)
                    nc.vector.reciprocal_approx_fast(rf[:], st[:])
                    with nc.allow_low_precision(
                        reason="1/s broadcast in bf16; ~0.4% noise ok"
                    ):
                        nc.vector.tensor_copy(
                            rs_all[0:1, p, ib, :, :].rearrange("a l c -> a (l c)"),
                            rf[:],
                        )
                    for lh in range(2):
                        nc.vector.tensor_copy(yTu[:, ib * 2 + lh, :], yT_ps[lh][0:D, :])

                def sm_pair(ib, pp):
                    """broadcast 1/s (gpsimd partition bcast), scale into yTn."""
                    for lh in range(2):
                        Sb = npool.tile([64, 512], BF16, tag="Sb", name="Sb")
                        nc.gpsimd.partition_broadcast(
                            Sb[:], rs_all[0:1, pp, ib, lh, :]
                        )
                        nc.vector.tensor_tensor(
                            yTn_s[64 * lh : 64 * lh + 64, pp, 512 * ib : 512 * ib + 512],
                            yTu_by_p[pp][:, ib * 2 + lh, :],
                            Sb[:],
                            mybir.AluOpType.mult,
                        )

                def outproj(ib):
                    ops = ops_box[0]
                    # finer DMA units spread over whichever queues are idle
                    # at that stage; 3-way fan-out for the final block so
                    # the trailing drain after the last cast is one hop
                    queues = [
                        [nc.sync, nc.gpsimd],
                        [nc.sync, nc.gpsimd],
                        [nc.sync, nc.scalar],
                        [nc.sync, nc.scalar, nc.gpsimd],
                    ][ib]
                    for tt in range(4 * ib, 4 * ib + 4):
                        ot = osb.tile([128, 1024], BF16, tag="out_t")
                        for nb in range(2):
                            o_ps = ops.tile([128, 512], F32, tag="out_ps", name="o_ps")
                            for pp in range(2):
                                nc.tensor.matmul(
                                    o_ps[:],
                                    yTn_s[:, pp, tt * 128 : (tt + 1) * 128],
                                    wp_s[:, pp, nb * 512 : (nb + 1) * 512],
                                    start=(pp == 0),
                                    stop=(pp == 1),
                                )
                            with nc.allow_low_precision(
                                reason="bf16 output partials; host sums in f32"
                            ):
                                nc.vector.tensor_copy(
                                    ot[:, nb * 512 : (nb + 1) * 512], o_ps[:]
                                )
                            q = queues[(tt * 2 + nb) % len(queues)]
                            q.dma_start(
                                out_r[tt, :, nb * 512 : (nb + 1) * 512],
                                ot[:, nb * 512 : (nb + 1) * 512],
                            )

                if p == 0:
                    for ib in range(IB):
                        _mark(nc, f"p0_att{ib}")
                        # rows 0-3 already emitted their first chunks in
                        # the projection tail
                        attT_rows(p, PT, range(4 * ib, 4 * ib + 4),
                                  off0=1024 if ib == 0 else 0)
                        _mark(nc, f"p0_fill{ib}")
                        fill((0, ib))
                        _mark(nc, f"p0_av{ib}")
                        yT_ps = av_block(ib)
                        stash_recip(ib, yT_ps)
                    # proj psum pool swaps out for the out-proj pool
                    pps_cm.__exit__(None, None, None)
                    ops_cm = tc.tile_pool(name="out_ps", bufs=2, space="PSUM")
                    ops_box[0] = ops_cm.__enter__()
                    fill_pool[0] = (ops_box[0], "out_ps")
                else:
                    # prologue: two i-blocks of attT ahead, first av staged;
                    # the late kT mt=1 groups land here (needed only from
                    # attT row 8 onward). Pair-0's scale-mults all drain
                    # here instead of serializing into the tail.
                    _mark(nc, "p1_att0")
                    attT_rows(p, PT, range(0, 4))
                    fill((1, -1))
                    for _ib in range(IB):
                        sm_pair(_ib, 0)
                    _mark(nc, "p1_av0")
                    yT_ps = av_block(0)
                    stash_recip(0, yT_ps)
                    _mark(nc, "p1_att1")
                    attT_rows(p, PT, range(4, 8))
                    for ib in range(IB):
                        _mark(nc, f"p1_sm{ib}")
                        sm_pair(ib, 1)
                        if ib + 1 < IB:
                            _mark(nc, f"p1_av{ib + 1}")
                            yT_ps = av_block(ib + 1)
                            stash_recip(ib + 1, yT_ps)
                        if ib == 0:
                            fill((1, 0))
                        if ib + 2 < IB:
                            _mark(nc, f"p1_att{ib + 2}")
                            attT_rows(p, PT, range(4 * (ib + 2), 4 * (ib + 2) + 4))
                        _mark(nc, f"p1_out{ib}")
                        outproj(ib)
                    ops_cm.__exit__(None, None, None)



_NC_CACHE = None


def get_nc() -> bass.Bass:
    global _NC_CACHE
    if _NC_CACHE is None:
        nc = bacc.Bacc()
        xT_d = nc.declare_dram_parameter("xT", [IB, C, 512], BF16, isOutput=False)
        wq_d = nc.declare_dram_parameter("wq", [2, C, 128], BF16, isOutput=False)
        wk_d = nc.declare_dram_parameter("wk", [2, C, 128], BF16, isOutput=False)
        wv_d = nc.declare_dram_parameter("wv", [C, MH], BF16, isOutput=False)
        wp_d = nc.declare_dram_parameter("wp", [MH, C], BF16, isOutput=False)
        bq_d = nc.declare_dram_parameter("bq", [MH], F32, isOutput=False)
        bk_d = nc.declare_dram_parameter("bk", [MH], F32, isOutput=False)
        out_d = nc.declare_dram_parameter("out", [T, C], BF16, isOutput=True)
        mask_d = nc.inline_tensor(_causal_mask(), name="causal_mask")
        emit_kernel(
            nc, xT_d, wq_d, wk_d, wv_d, wp_d, bq_d, bk_d, out_d, mask_d
        )
        nc.finalize()
        _NC_CACHE = nc
    return _NC_CACHE


def make_in_maps(x, Wq, bq, Wk, bk, Wv, bv, Wp, bp):
    in_maps = []
    for core in range(N_CORES):
        b, g = divmod(core, GROUPS)
        sl = slice(g * MH, (g + 1) * MH)
        in_maps.append(
            {
                "xT": np.ascontiguousarray(
                    x[b].T.reshape(C, 4, 512).transpose(1, 0, 2)
                ).astype(NP_BF16),
                "wq": np.ascontiguousarray(
                    np.stack([Wq[:, sl][:, 0:128], Wq[:, sl][:, 128:MH]])
                ).astype(NP_BF16),
                "wk": np.ascontiguousarray(
                    np.stack([Wk[:, sl][:, 0:128], Wk[:, sl][:, 128:MH]])
                ).astype(NP_BF16),
                "wv": np.ascontiguousarray(Wv[:, sl]).astype(NP_BF16),
                "wp": np.ascontiguousarray(Wp[sl, :]).astype(NP_BF16),
                "bq": np.ascontiguousarray(bq[sl]).astype(np.float32),
                "bk": np.ascontiguousarray(bk[sl]).astype(np.float32),
            }
        )
    return in_maps


def kernel(x, Wq, bq, Wk, bk, Wv, bv, Wp, bp, _results_hook=None, _trace=False):
    x = np.asarray(x, dtype=np.float32)
    nc = get_nc()
    in_maps = make_in_maps(x, Wq, bq, Wk, bk, Wv, bv, Wp, bp)
    res = run_bass_kernel_spmd(
        nc, in_maps, core_ids=list(range(N_CORES)), trace=_trace
    )
    if _results_hook is not None:
        _results_hook(res)
    out = np.zeros((B, T, C), dtype=np.float32)
    for core in range(N_CORES):
        b = core // GROUPS
        out[b] += np.asarray(res.results[core]["out"], dtype=np.float32)
    # v-bias folds through softmax exactly (attention rows sum to 1):
    # y = att @ (v + 1 bv^T)  =>  out += bv @ Wp, plus the output bias bp
    bias_row = (
        np.asarray(bv, dtype=np.float32) @ np.asarray(Wp, dtype=np.float32)
        + np.asarray(bp, dtype=np.float32)
    )
    out += bias_row[None, None, :]
    return out



# revision 32
# speedup vs baseline: 1.0080x; 1.0080x over previous
"""Causal self-attention Trainium2 kernel (8 NeuronCores, SPMD).

Sharding: 8 cores = 2 batches x 4 head-groups (4 heads of 64 dims each).
Each core computes full-sequence attention for its 4 heads plus the
partial output projection for its 256 y-columns; the host sums the 4
partials per batch and adds the (bv @ Wp + bp) bias row — the v-bias
folds through softmax exactly since attention rows sum to 1.

Layout strategy (no on-device transposes anywhere):
  - host supplies x[b].T t-chunk-major as xT [4, C, 512] (bf16) and
    wq/wk column-halved as [2, C, 128]: every input DMA moves fully
    contiguous >=2KB-per-partition descriptor lines (strided descriptors
    measured ~50GB/s vs ~95GB/s/queue contiguous); chunk tb=0 is split
    3-ways across the sync/scalar/gpsimd queues so the first projection
    starts ~13us in, with early dummy PE matmuls holding the p-state up
  - qT, kT produced in [m, t] layout (W stationary, xT moving); the
    PSUM->SBUF copy runs on the then-idle scalar engine as an Identity
    activation with the per-partition bias AP (exp/identity/copy share
    one activation table, so no ACT_TABLE_LOAD thrash)
  - v produced in natural [t, m] layout, augmented with a ones column
    per head (M=65) so the attention-value matmul also emits the
    softmax denominator row for free
  - attT[j, i] = sum_d kT[d,j] qT[d,i]  (kT stationary K=64; two heads
    run concurrently via row-tiled base partitions 0/64)
  - exp on ScalarE (fused 1/sqrt(64) scale) in 1024-wide chunks;
    diagonal 128-tiles masked with a multiplicative mask tile on DVE
  - PT rows stored triangularly (only i >= 128*jt) — fits in SBUF
    alongside xT, letting the second head-pair's q/k projections and
    the deferred v tiles interleave into pair 0's scalar-bound stretch
  - 1/s via DVE reciprocal_approx_fast, broadcast across partitions on
    the otherwise-idle GpSimd engine (partition_broadcast) — keeps the
    PE free of K=1 broadcast matmuls; pair-0's scale-mults all drain at
    pair-1's prologue so only pair-1's two remain on the tail
  - pair-1 pipeline: scale-mults for i-block N run while the
    attention-value pass for N+1 runs on the PE; out-proj follows
  - out[t, n] partials bf16, DMA'd per 512-column half on whichever
    queues are idle at that stage (gpsimd mid-run, scalar after exp
    ends, 3-way fan-out for the last block) to shrink the drain tail
  - pair-0's first score rows + exps are emitted inside the projection
    tail so the ~83us scalar exp stream (the saturated engine through
    the middle) starts ~25us in; late phase-1 copies route to DVE so
    they never extend the scalar stream
  - power note: HAM clock-gates the PE (k=8 -> k=4) under sustained
    dense multi-engine activity; a FULLY-interleaved schedule measured
    slower purely from added throttle — this lite interleave is the
    measured sweet spot
"""

import sys

for _p in ("/opt/trn_rl_repo",):
    if _p not in sys.path:
        sys.path.insert(0, _p)

from contextlib import ExitStack

import ml_dtypes
import numpy as np

import concourse.bass as bass
import concourse.tile as tile
from concourse import bacc, mybir
from concourse.bass_utils import run_bass_kernel_spmd

BF16 = mybir.dt.bfloat16
F32 = mybir.dt.float32
NP_BF16 = ml_dtypes.bfloat16

PHASE_MARKS = []


def _mark(nc, label):
    nm = nc.get_next_instruction_name()  # burn one name as a phase boundary
    PHASE_MARKS.append((label, int(nm.split("-")[1])))


B, T, C = 2, 2048, 1024
H, D = 16, 64
N_CORES = 8
GROUPS = 4          # head groups (cores per batch)
MH = C // GROUPS    # 256 columns per core (4 heads)
LH = MH // D        # 4 local heads
CT = C // 128       # 8 contraction tiles
TT = T // 128       # 16 sequence tiles of 128
IB = T // 512       # 4 i-blocks of 512
SCALE = 1.0 / np.sqrt(D)
N_WARM = 16         # dummy matmuls covering the initial DMA wait

# triangular PT layout: row jt holds cols i in [128*jt, T)
TRI_OFF = [0] * TT
for _jt in range(1, TT):
    TRI_OFF[_jt] = TRI_OFF[_jt - 1] + (T - 128 * (_jt - 1))
TRI = TRI_OFF[-1] + (T - 128 * (TT - 1))   # 17408


def _causal_mask() -> np.ndarray:
    """mask[j, i] = 1.0 if j <= i else 0 (bf16), [128, 128]."""
    j = np.arange(128)[:, None]
    i = np.arange(128)[None, :]
    return (j <= i).astype(NP_BF16)


def emit_kernel(nc, xT_d, wq_d, wk_d, wv_d, wp_d, bq_d, bk_d, out_d, mask_d):
    with tile.TileContext(nc) as tc, ExitStack() as ctx:
        _mark(nc, "setup")
        # ---- long-lived tiles -------------------------------------------
        keep = ctx.enter_context(tc.tile_pool(name="keep", bufs=1))
        qT_s = keep.tile([128, 2, T], BF16, tag="qT")
        kT_s = keep.tile([128, 2, T], BF16, tag="kT")
        v_s = keep.tile([128, TT, LH, D + 1], BF16, tag="v")
        yTn_s = keep.tile([128, 2, T], BF16, tag="yTn")
        wp_s = keep.tile([128, 2, C], BF16, tag="wp")
        mask_st = keep.tile([128, 128], BF16, tag="mask_st")
        mask_s = keep.tile([128, 128], BF16, tag="mask")
        bq_st = keep.tile([128, 2], F32, tag="bq_st")
        bq_s = keep.tile([128, 2], F32, tag="bq")
        bk_st = keep.tile([128, 2], F32, tag="bk_st")
        bk_s = keep.tile([128, 2], F32, tag="bk")
        rs_all = keep.tile([1, 2, IB, 2, 512], BF16, tag="rs_all")  # 1/s rows
        warm_s = keep.tile([128, 256], BF16, tag="warm")

        nc.vector.memset(warm_s[:], 1.0)

        # PE warm-up first thing: dummy matmuls keep the p-state ramping
        # while the input DMAs land (PE queue is in-order, so these retire
        # before the first projection without delaying it)
        with tc.tile_pool(name="warm_ps", bufs=1, space="PSUM") as wpool:
            for _ in range(N_WARM):
                wps_t = wpool.tile([128, 512], F32, tag="warm_ps")
                nc.tensor.matmul(
                    wps_t[:, 0:256], warm_s[:, 0:128], warm_s[:], start=True, stop=True
                )

        # projection inputs stay alive through pair 0's attention so the
        # mt=1 q/k projections can interleave there
        pin = ctx.enter_context(tc.tile_pool(name="proj_in", bufs=1))
        xT_s = pin.tile([128, IB, CT, 512], BF16, tag="xT")  # t-chunk major
        wq_s = pin.tile([128, CT, MH], BF16, tag="wq")
        wk_s = pin.tile([128, CT, MH], BF16, tag="wk")
        wv_s = pin.tile([128, CT, MH], BF16, tag="wv")
        xT_r = xT_d.ap().rearrange("c (p o) t -> c p o t", p=128)
        wq_r = wq_d.ap().rearrange("h (p o) m -> h p o m", p=128)
        wk_r = wk_d.ap().rearrange("h (p o) m -> h p o m", p=128)
        wv_r = wv_d.ap().rearrange("(p o) m -> p o m", p=128)

        # consts first (tiny, land immediately), staged through a DVE copy:
        # consumers then depend on DVE program order instead of a DMA
        # semaphore (walrus 1-wait limit)
        # xT arrives t-chunk-major ([4, C, 512] in DRAM, chunk-major SBUF
        # tile): fully-contiguous 4KB-per-partition descriptor lines. The
        # first chunk (tb=0) is split three ways across all hwdge queues so
        # the first projection group can start ~13us in; later chunks
        # alternate sync/scalar halves.
        nc.sync.dma_start(wq_s[:, :, 0:128], wq_r[0])
        nc.scalar.dma_start(xT_s[:, 0, 0:3, :], xT_r[0, :, 0:3, :])
        nc.gpsimd.dma_start(xT_s[:, 0, 3:6, :], xT_r[0, :, 3:6, :])
        nc.sync.dma_start(xT_s[:, 0, 6:CT, :], xT_r[0, :, 6:CT, :])
        nc.gpsimd.dma_start(wk_s[:, :, 0:128], wk_r[0])
        nc.gpsimd.dma_start(mask_st[:], mask_d.ap())
        nc.gpsimd.dma_start(bq_st[:], bq_d.ap().rearrange("(o p) -> p o", p=128))
        nc.gpsimd.dma_start(bk_st[:], bk_d.ap().rearrange("(o p) -> p o", p=128))
        for _tb in range(1, IB):
            nc.sync.dma_start(xT_s[:, _tb, 0:4, :], xT_r[_tb, :, 0:4, :])
            nc.scalar.dma_start(xT_s[:, _tb, 4:CT, :], xT_r[_tb, :, 4:CT, :])
            if _tb == 1:
                nc.gpsimd.dma_start(wv_s[:], wv_r[:])
        nc.gpsimd.dma_start(wq_s[:, :, 128:MH], wq_r[1])
        nc.gpsimd.dma_start(wk_s[:, :, 128:MH], wk_r[1])
        wp_r = wp_d.ap().rearrange("(o p) n -> p o n", p=128)
        nc.gpsimd.dma_start(wp_s[:], wp_r[:])
        nc.vector.tensor_copy(mask_s[:], mask_st[:])
        nc.vector.tensor_copy(bq_s[:], bq_st[:])
        nc.vector.tensor_copy(bk_s[:], bk_st[:])
        nc.vector.memset(v_s[:, :, :, D : D + 1], 1.0)

        def proj_group(ps, w_s, b_s, dst, mt, tb, on_scalar=True):
            """one [128, 512] column block of qT or kT (8-deep K accum)."""
            for ct in range(CT):
                nc.tensor.matmul(
                    ps[:, 0:512],
                    w_s[:, ct, mt * 128 : (mt + 1) * 128],
                    xT_s[:, tb, ct, :],
                    start=(ct == 0),
                    stop=(ct == CT - 1),
                )
            if on_scalar:
                nc.scalar.activation(
                    dst[:, mt, tb * 512 : (tb + 1) * 512],
                    ps[:, 0:512],
                    mybir.ActivationFunctionType.Identity,
                    bias=b_s[:, mt : mt + 1],
                )
            else:
                nc.vector.tensor_scalar(
                    dst[:, mt, tb * 512 : (tb + 1) * 512],
                    ps[:, 0:512],
                    b_s[:, mt : mt + 1],
                    None,
                    mybir.AluOpType.add,
                )

        def v_group(tt, ps, on_scalar=True):
            # v natural [t, m]  (xT stationary)
            for ct in range(CT):
                nc.tensor.matmul(
                    ps[:, 0:MH],
                    xT_s[:, tt // 4, ct, (tt % 4) * 128 : (tt % 4) * 128 + 128],
                    wv_s[:, ct, :],
                    start=(ct == 0),
                    stop=(ct == CT - 1),
                )
            if on_scalar:
                nc.scalar.activation(
                    v_s[:, tt, :, 0:D],
                    ps[:, 0:MH].rearrange("p (h d) -> p h d", h=LH),
                    mybir.ActivationFunctionType.Copy,
                )
            else:
                nc.vector.tensor_copy(
                    v_s[:, tt, :, 0:D],
                    ps[:, 0:MH].rearrange("p (h d) -> p h d", h=LH),
                )

        # ---- attention with early-start exp -----------------------------
        # Scalar (exp) is the saturated engine through the middle of the
        # run, so the first score chunks + exps are emitted inside the
        # projection tail: the 83us exp stream starts ~17us earlier.
        # PSUM: att(4) + yt(2) + proj(2) banks during pair 0; the proj
        # pool swaps out for the out-proj pool at pair 1.
        with (
            tc.tile_pool(name="pt", bufs=1) as ptp,
            tc.tile_pool(name="norm", bufs=2) as npool,
            tc.tile_pool(name="out_sb", bufs=6) as osb,
            tc.tile_pool(name="yt_ps", bufs=2, space="PSUM") as yps,
        ):
            out_r = out_d.ap().rearrange("(tt p) n -> tt p n", p=128)
            yTu_by_p = [None, None]
            aps_cm = tc.tile_pool(name="att_ps", bufs=2, space="PSUM")
            aps = aps_cm.__enter__()
            pps_cm = tc.tile_pool(name="proj_ps", bufs=2, space="PSUM")
            pps = pps_cm.__enter__()
            fill_pool = [(pps, "proj_ps")]
            ops_box = [None]

            def v_group(tt, ps, on_scalar=False):
                # v natural [t, m]  (xT stationary)
                for ct in range(CT):
                    nc.tensor.matmul(
                        ps[:, 0:MH],
                        xT_s[:, tt // 4, ct, (tt % 4) * 128 : (tt % 4) * 128 + 128],
                        wv_s[:, ct, :],
                        start=(ct == 0),
                        stop=(ct == CT - 1),
                    )
                if on_scalar:
                    nc.scalar.activation(
                        v_s[:, tt, :, 0:D],
                        ps[:, 0:MH].rearrange("p (h d) -> p h d", h=LH),
                        mybir.ActivationFunctionType.Copy,
                    )
                else:
                    nc.vector.tensor_copy(
                        v_s[:, tt, :, 0:D],
                        ps[:, 0:MH].rearrange("p (h d) -> p h d", h=LH),
                    )

            def attT_chunk(p, PT, jt, off):
                """attT + exp (+ diag mask) for row jt, cols off..off+1024."""
                ia = 128 * jt
                base = TRI_OFF[jt]
                cw = min(1024, T - ia - off)
                for lh in range(2):
                    att_ps = aps.tile([128, 1024], F32, tag="att_ps")
                    prow = slice(64 * lh, 64 * lh + 64)
                    for s5 in range(0, cw, 512):
                        nn = min(512, cw - s5)
                        nc.tensor.matmul(
                            att_ps[:, s5 : s5 + nn],
                            kT_s[prow, p, jt * 128 : (jt + 1) * 128],
                            qT_s[prow, p, ia + off + s5 : ia + off + s5 + nn],
                            start=True,
                            stop=True,
                        )
                    nc.scalar.activation(
                        PT[lh][:, base + off : base + off + cw],
                        att_ps[:, :cw],
                        mybir.ActivationFunctionType.Exp,
                        scale=float(SCALE),
                    )
                    if off == 0:
                        # diagonal 128x128 tile: zero j > i
                        nc.vector.tensor_tensor(
                            PT[lh][:, base : base + 128],
                            PT[lh][:, base : base + 128],
                            mask_s[:],
                            mybir.AluOpType.mult,
                        )

            def attT_rows(p, PT, jts, off0=0):
                """chunk-major over a group of rows: all chunk0 first."""
                offs = [off0] * len(jts)
                while True:
                    emitted_any = False
                    for k, jt in enumerate(jts):
                        if offs[k] < T - 128 * jt:
                            attT_chunk(p, PT, jt, offs[k])
                            offs[k] += 1024
                            emitted_any = True
                    if not emitted_any:
                        break

            def new_PT():
                return [
                    ptp.tile([128, TRI], BF16, tag=f"PT{lh}", name=f"PT{lh}")
                    for lh in range(2)
                ]

            # ---- phase 1: mt=0 projections + v, pair-0 scores start -----
            PT0 = new_PT()
            _mark(nc, "proj_qkv")
            for tb in range(IB):
                late = tb >= 2   # scalar is exp-busy from here on
                for w_s, b_s, dst in ((wq_s, bq_s, qT_s), (wk_s, bk_s, kT_s)):
                    ps = pps.tile([128, 512], F32, tag="proj_ps", name="proj_ps")
                    proj_group(ps, w_s, b_s, dst, 0, tb, on_scalar=not late)
                if tb == 1:
                    for tt in (0, 1):
                        ps = pps.tile([128, 512], F32, tag="proj_ps", name="v_ps")
                        v_group(tt, ps, on_scalar=True)
                elif tb == 2:
                    attT_chunk(0, PT0, 0, 0)
                    attT_chunk(0, PT0, 1, 0)
                    for tt in (2, 3):
                        ps = pps.tile([128, 512], F32, tag="proj_ps", name="v_ps")
                        v_group(tt, ps)
                elif tb == 3:
                    attT_chunk(0, PT0, 2, 0)
                    attT_chunk(0, PT0, 3, 0)
                    for tt in (4, 5, 6, 7):
                        ps = pps.tile([128, 512], F32, tag="proj_ps", name="v_ps")
                        v_group(tt, ps)

            # PE filler work for the scalar-bound attention stretches:
            # mt=1 q/k projection groups and the deferred v tiles 8-15
            def _mt1(w_s, b_s, dst, tb):
                def emit():
                    pool, tag = fill_pool[0]
                    ps = pool.tile([128, 512], F32, tag=tag, name="fill_ps")
                    proj_group(ps, w_s, b_s, dst, 1, tb, on_scalar=False)
                return emit

            def _vg(tt):
                def emit():
                    pool, tag = fill_pool[0]
                    ps = pool.tile([128, 512], F32, tag=tag, name="fill_ps")
                    v_group(tt, ps)
                return emit

            fill_sched = {
                (0, 0): [_mt1(wq_s, bq_s, qT_s, 0), _mt1(wq_s, bq_s, qT_s, 1),
                         _vg(8), _vg(9)],
                (0, 1): [_mt1(wq_s, bq_s, qT_s, 2), _mt1(wq_s, bq_s, qT_s, 3),
                         _vg(10), _vg(11)],
                (0, 2): [_mt1(wk_s, bk_s, kT_s, 0), _vg(12), _vg(13)],
                (0, 3): [_mt1(wk_s, bk_s, kT_s, 1), _vg(14), _vg(15)],
                (1, -1): [_mt1(wk_s, bk_s, kT_s, 2)],
                (1, 0): [_mt1(wk_s, bk_s, kT_s, 3)],
            }

            def fill(key):
                for emit in fill_sched.pop(key, []):
                    emit()

            for p in range(2):
                PT = PT0 if p == 0 else new_PT()
                yTu = npool.tile([64, 8, 512], BF16, tag="yTu", name="yTu")
                yTu_by_p[p] = yTu

                def av_block(ib):
                    """attention @ v for i-block ib; returns yT_ps pair."""
                    yT_ps = [
                        yps.tile([D + 1, 512], F32, tag="yT_ps", name=f"yT_ps{lh}")
                        for lh in range(2)
                    ]
                    for jt in range(4 * ib + 4):
                        for lh in range(2):
                            ia = 128 * jt
                            c0 = max(512 * ib, ia)
                            nc.tensor.matmul(
                                yT_ps[lh][:, c0 - 512 * ib : 512],
                                v_s[:, jt, 2 * p + lh, :],
                                PT[lh][
                                    :,
                                    TRI_OFF[jt]
                                    + c0
                                    - ia : TRI_OFF[jt]
                                    + 512 * ib
                                    + 512
                                    - ia,
                                ],
                                start=(jt == 0),
                                stop=(jt == 4 * ib + 3),
                            )
                    return yT_ps

                def stash_recip(ib, yT_ps):
                    """denominators + 1/s first (they gate s_mults), then y."""
                    st = npool.tile([1, 1024], F32, tag="st", name="st", bufs=1)
                    for lh in range(2):
                        nc.vector.tensor_copy(
                            st[0:1, lh * 512 : (lh + 1) * 512], yT_ps[lh][D : D + 1, :]
                        )
                    rf = npool.tile([1, 1024], F32, tag="rf", name="rf", bufs=1)
                    nc.vector.reciprocal_approx_fast(rf[:], st[:])
                    with nc.allow_low_precision(
                        reason="1/s broadcast in bf16; ~0.4% noise ok"
                    ):
                        nc.vector.tensor_copy(
                            rs_all[0:1, p, ib, :, :].rearrange("a l c -> a (l c)"),
                            rf[:],
                        )
                    for lh in range(2):
                        nc.vector.tensor_copy(yTu[:, ib * 2 + lh, :], yT_ps[lh][0:D, :])

                def sm_pair(ib, pp):
                    """broadcast 1/s (gpsimd partition bcast), scale into yTn."""
                    for lh in range(2):
                        Sb = npool.tile([64, 512], BF16, tag="Sb", name="Sb")
                        nc.gpsimd.partition_broadcast(
                            Sb[:], rs_all[0:1, pp, ib, lh, :]
                        )
                        nc.vector.tensor_tensor(
                            yTn_s[64 * lh : 64 * lh + 64, pp, 512 * ib : 512 * ib + 512],
                            yTu_by_p[pp][:, ib * 2 + lh, :],
                            Sb[:],
                            mybir.AluOpType.mult,
                        )

                def outproj(ib):
                    ops = ops_box[0]
                    # finer DMA units spread over whichever queues are idle
                    # at that stage; 3-way fan-out for the final block so
                    # the trailing drain after the last cast is one hop
                    queues = [
                        [nc.sync, nc.gpsimd],
                        [nc.sync, nc.gpsimd],
                        [nc.sync, nc.scalar],
                        [nc.sync, nc.scalar, nc.gpsimd],
                    ][ib]
                    for tt in range(4 * ib, 4 * ib + 4):
                        ot = osb.tile([128, 1024], BF16, tag="out_t")
                        for nb in range(2):
                            o_ps = ops.tile([128, 512], F32, tag="out_ps", name="o_ps")
                            for pp in range(2):
                                nc.tensor.matmul(
                                    o_ps[:],
                                    yTn_s[:, pp, tt * 128 : (tt + 1) * 128],
                                    wp_s[:, pp, nb * 512 : (nb + 1) * 512],
                                    start=(pp == 0),
                                    stop=(pp == 1),
                                )
                            with nc.allow_low_precision(
                                reason="bf16 output partials; host sums in f32"
                            ):
                                nc.vector.tensor_copy(
                                    ot[:, nb * 512 : (nb + 1) * 512], o_ps[:]
                                )
                            q = queues[(tt * 2 + nb) % len(queues)]
                            q.dma_start(
                                out_r[tt, :, nb * 512 : (nb + 1) * 512],
                                ot[:, nb * 512 : (nb + 1) * 512],
                            )

                if p == 0:
                    for ib in range(IB):
                        _mark(nc, f"p0_att{ib}")
                        # rows 0-3 already emitted their first chunks in
                        # the projection tail
                        attT_rows(p, PT, range(4 * ib, 4 * ib + 4),
                                  off0=1024 if ib == 0 else 0)
                        _mark(nc, f"p0_fill{ib}")
                        fill((0, ib))
                        _mark(nc, f"p0_av{ib}")
                        yT_ps = av_block(ib)
                        stash_recip(ib, yT_ps)
                    # proj psum pool swaps out for the out-proj pool
                    pps_cm.__exit__(None, None, None)
                    ops_cm = tc.tile_pool(name="out_ps", bufs=2, space="PSUM")
                    ops_box[0] = ops_cm.__enter__()
                    fill_pool[0] = (ops_box[0], "out_ps")
                else:
                    # prologue: two i-blocks of attT ahead, first av staged;
                    # the late kT mt=1 groups land here (needed only from
                    # attT row 8 onward). Pair-0's scale-mults all drain
                    # here instead of serializing into the tail.
                    _mark(nc, "p1_att0")
                    attT_rows(p, PT, range(0, 4))
                    fill((1, -1))
                    for _ib in range(IB):
                        sm_pair(_ib, 0)
                    _mark(nc, "p1_av0")
                    yT_ps = av_block(0)
                    stash_recip(0, yT_ps)
                    _mark(nc, "p1_att1")
                    attT_rows(p, PT, range(4, 8))
                    for ib in range(IB):
                        _mark(nc, f"p1_sm{ib}")
                        sm_pair(ib, 1)
                        if ib + 1 < IB:
                            _mark(nc, f"p1_av{ib + 1}")
                            yT_ps = av_block(ib + 1)
                            stash_recip(ib + 1, yT_ps)
                        if ib == 0:
                            fill((1, 0))
                        if ib + 2 < IB:
                            _mark(nc, f"p1_att{ib + 2}")
                            attT_rows(p, PT, range(4 * (ib + 2), 4 * (ib + 2) + 4))
                        if ib == 1:
                            # last score chunk emitted: the att psum pool is
                            # dead — swap in a 4-buf out-proj pool so the
                            # matmul/cast tail pipelines without bubbles
                            ops_cm.__exit__(None, None, None)
                            aps_cm.__exit__(None, None, None)
                            ops_cm = tc.tile_pool(
                                name="out_ps2", bufs=4, space="PSUM"
                            )
                            ops_box[0] = ops_cm.__enter__()
                        _mark(nc, f"p1_out{ib}")
                        outproj(ib)
                    ops_cm.__exit__(None, None, None)



_NC_CACHE = None


def get_nc() -> bass.Bass:
    global _NC_CACHE
    if _NC_CACHE is None:
        nc = bacc.Bacc()
        xT_d = nc.declare_dram_parameter("xT", [IB, C, 512], BF16, isOutput=False)
        wq_d = nc.declare_dram_parameter("wq", [2, C, 128], BF16, isOutput=False)
        wk_d = nc.declare_dram_parameter("wk", [2, C, 128], BF16, isOutput=False)
        wv_d = nc.declare_dram_parameter("wv", [C, MH], BF16, isOutput=False)
        wp_d = nc.declare_dram_parameter("wp", [MH, C], BF16, isOutput=False)
        bq_d = nc.declare_dram_parameter("bq", [MH], F32, isOutput=False)
        bk_d = nc.declare_dram_parameter("bk", [MH], F32, isOutput=False)
        out_d = nc.declare_dram_parameter("out", [T, C], BF16, isOutput=True)
        mask_d = nc.inline_tensor(_causal_mask(), name="causal_mask")
        emit_kernel(
            nc, xT_d, wq_d, wk_d, wv_d, wp_d, bq_d, bk_d, out_d, mask_d
        )
        nc.finalize()
        _NC_CACHE = nc
    return _NC_CACHE


def make_in_maps(x, Wq, bq, Wk, bk, Wv, bv, Wp, bp):
    in_maps = []
    for core in range(N_CORES):
        b, g = divmod(core, GROUPS)
        sl = slice(g * MH, (g + 1) * MH)
        in_maps.append(
            {
                "xT": np.ascontiguousarray(
                    x[b].T.reshape(C, 4, 512).transpose(1, 0, 2)
                ).astype(NP_BF16),
                "wq": np.ascontiguousarray(
                    np.stack([Wq[:, sl][:, 0:128], Wq[:, sl][:, 128:MH]])
                ).astype(NP_BF16),
                "wk": np.ascontiguousarray(
                    np.stack([Wk[:, sl][:, 0:128], Wk[:, sl][:, 128:MH]])
                ).astype(NP_BF16),
                "wv": np.ascontiguousarray(Wv[:, sl]).astype(NP_BF16),
                "wp": np.ascontiguousarray(Wp[sl, :]).astype(NP_BF16),
                "bq": np.ascontiguousarray(bq[sl]).astype(np.float32),
                "bk": np.ascontiguousarray(bk[sl]).astype(np.float32),
            }
        )
    return in_maps


def kernel(x, Wq, bq, Wk, bk, Wv, bv, Wp, bp, _results_hook=None, _trace=False):
    x = np.asarray(x, dtype=np.float32)
    nc = get_nc()
    in_maps = make_in_maps(x, Wq, bq, Wk, bk, Wv, bv, Wp, bp)
    res = run_bass_kernel_spmd(
        nc, in_maps, core_ids=list(range(N_CORES)), trace=_trace
    )
    if _results_hook is not None:
        _results_hook(res)
    out = np.zeros((B, T, C), dtype=np.float32)
    for core in range(N_CORES):
        b = core // GROUPS
        out[b] += np.asarray(res.results[core]["out"], dtype=np.float32)
    # v-bias folds through softmax exactly (attention rows sum to 1):
    # y = att @ (v + 1 bv^T)  =>  out += bv @ Wp, plus the output bias bp
    bias_row = (
        np.asarray(bv, dtype=np.float32) @ np.asarray(Wp, dtype=np.float32)
        + np.asarray(bp, dtype=np.float32)
    )
    out += bias_row[None, None, :]
    return out



# revision 33
# speedup vs baseline: 1.0146x; 1.0065x over previous
"""Causal self-attention Trainium2 kernel (8 NeuronCores, SPMD).

Sharding: 8 cores = 2 batches x 4 head-groups (4 heads of 64 dims each).
Each core computes full-sequence attention for its 4 heads plus the
partial output projection for its 256 y-columns; the host sums the 4
partials per batch and adds the (bv @ Wp + bp) bias row — the v-bias
folds through softmax exactly since attention rows sum to 1.

Layout strategy (no on-device transposes anywhere):
  - host supplies x[b].T t-chunk-major as xT [4, C, 512] (bf16) and
    wq/wk column-halved as [2, C, 128]: every input DMA moves fully
    contiguous >=2KB-per-partition descriptor lines (strided descriptors
    measured ~50GB/s vs ~95GB/s/queue contiguous); chunk tb=0 is split
    3-ways across the sync/scalar/gpsimd queues so the first projection
    starts ~13us in, with early dummy PE matmuls holding the p-state up
  - qT, kT produced in [m, t] layout (W stationary, xT moving); the
    PSUM->SBUF copy runs on the then-idle scalar engine as an Identity
    activation with the per-partition bias AP (exp/identity/copy share
    one activation table, so no ACT_TABLE_LOAD thrash)
  - v produced in natural [t, m] layout, augmented with a ones column
    per head (M=65) so the attention-value matmul also emits the
    softmax denominator row for free
  - attT[j, i] = sum_d kT[d,j] qT[d,i]  (kT stationary K=64; two heads
    run concurrently via row-tiled base partitions 0/64)
  - exp on ScalarE (fused 1/sqrt(64) scale) in 1024-wide chunks;
    diagonal 128-tiles masked with a multiplicative mask tile on DVE
  - PT rows stored triangularly (only i >= 128*jt) — fits in SBUF
    alongside xT, letting the second head-pair's q/k projections and
    the deferred v tiles interleave into pair 0's scalar-bound stretch
  - 1/s via DVE reciprocal_approx_fast, broadcast across partitions on
    the otherwise-idle GpSimd engine (partition_broadcast) — keeps the
    PE free of K=1 broadcast matmuls; pair-0's scale-mults all drain at
    pair-1's prologue so only pair-1's two remain on the tail
  - pair-1 pipeline: scale-mults for i-block N run while the
    attention-value pass for N+1 runs on the PE; out-proj follows
  - out[t, n] partials bf16, DMA'd per 512-column half on whichever
    queues are idle at that stage (gpsimd mid-run, scalar after exp
    ends, 3-way fan-out for the last block) to shrink the drain tail
  - pair-0's first score rows + exps are emitted inside the projection
    tail so the ~83us scalar exp stream (the saturated engine through
    the middle) starts ~25us in; late phase-1 copies route to DVE so
    they never extend the scalar stream
  - power note: HAM clock-gates the PE (k=8 -> k=4) under sustained
    dense multi-engine activity; a FULLY-interleaved schedule measured
    slower purely from added throttle — this lite interleave is the
    measured sweet spot
"""

import sys

for _p in ("/opt/trn_rl_repo",):
    if _p not in sys.path:
        sys.path.insert(0, _p)

from contextlib import ExitStack

import ml_dtypes
import numpy as np

import concourse.bass as bass
import concourse.tile as tile
from concourse import bacc, mybir
from concourse.bass_utils import run_bass_kernel_spmd

BF16 = mybir.dt.bfloat16
F32 = mybir.dt.float32
NP_BF16 = ml_dtypes.bfloat16

PHASE_MARKS = []


def _mark(nc, label):
    nm = nc.get_next_instruction_name()  # burn one name as a phase boundary
    PHASE_MARKS.append((label, int(nm.split("-")[1])))


B, T, C = 2, 2048, 1024
H, D = 16, 64
N_CORES = 8
GROUPS = 4          # head groups (cores per batch)
MH = C // GROUPS    # 256 columns per core (4 heads)
LH = MH // D        # 4 local heads
CT = C // 128       # 8 contraction tiles
TT = T // 128       # 16 sequence tiles of 128
IB = T // 512       # 4 i-blocks of 512
SCALE = 1.0 / np.sqrt(D)
N_WARM = 16         # dummy matmuls covering the initial DMA wait

# triangular PT layout: row jt holds cols i in [128*jt, T)
TRI_OFF = [0] * TT
for _jt in range(1, TT):
    TRI_OFF[_jt] = TRI_OFF[_jt - 1] + (T - 128 * (_jt - 1))
TRI = TRI_OFF[-1] + (T - 128 * (TT - 1))   # 17408


def _causal_mask() -> np.ndarray:
    """mask[j, i] = 1.0 if j <= i else 0 (bf16), [128, 128]."""
    j = np.arange(128)[:, None]
    i = np.arange(128)[None, :]
    return (j <= i).astype(NP_BF16)


def emit_kernel(nc, xT_d, wq_d, wk_d, wv_d, wp_d, bq_d, bk_d, out_d, mask_d):
    with tile.TileContext(nc) as tc, ExitStack() as ctx:
        _mark(nc, "setup")
        # ---- long-lived tiles -------------------------------------------
        keep = ctx.enter_context(tc.tile_pool(name="keep", bufs=1))
        qT_s = keep.tile([128, 2, T], BF16, tag="qT")
        kT_s = keep.tile([128, 2, T], BF16, tag="kT")
        v_s = keep.tile([128, TT, LH, D + 1], BF16, tag="v")
        yTn_s = keep.tile([128, 2, T], BF16, tag="yTn")
        wp_s = keep.tile([128, 2, C], BF16, tag="wp")
        mask_st = keep.tile([128, 128], BF16, tag="mask_st")
        mask_s = keep.tile([128, 128], BF16, tag="mask")
        bq_st = keep.tile([128, 2], F32, tag="bq_st")
        bq_s = keep.tile([128, 2], F32, tag="bq")
        bk_st = keep.tile([128, 2], F32, tag="bk_st")
        bk_s = keep.tile([128, 2], F32, tag="bk")
        rs_all = keep.tile([1, 2, IB, 2, 512], BF16, tag="rs_all")  # 1/s rows
        warm_s = keep.tile([128, 256], BF16, tag="warm")

        nc.vector.memset(warm_s[:], 1.0)

        # PE warm-up first thing: dummy matmuls keep the p-state ramping
        # while the input DMAs land (PE queue is in-order, so these retire
        # before the first projection without delaying it)
        with tc.tile_pool(name="warm_ps", bufs=1, space="PSUM") as wpool:
            for _ in range(N_WARM):
                wps_t = wpool.tile([128, 512], F32, tag="warm_ps")
                nc.tensor.matmul(
                    wps_t[:, 0:256], warm_s[:, 0:128], warm_s[:], start=True, stop=True
                )

        # projection inputs stay alive through pair 0's attention so the
        # mt=1 q/k projections can interleave there
        pin = ctx.enter_context(tc.tile_pool(name="proj_in", bufs=1))
        xT_s = pin.tile([128, IB, CT, 512], BF16, tag="xT")  # t-chunk major
        wq_s = pin.tile([128, CT, MH], BF16, tag="wq")
        wk_s = pin.tile([128, CT, MH], BF16, tag="wk")
        wv_s = pin.tile([128, CT, MH], BF16, tag="wv")
        xT_r = xT_d.ap().rearrange("c (p o) t -> c p o t", p=128)
        wq_r = wq_d.ap().rearrange("h (p o) m -> h p o m", p=128)
        wk_r = wk_d.ap().rearrange("h (p o) m -> h p o m", p=128)
        wv_r = wv_d.ap().rearrange("(p o) m -> p o m", p=128)

        # consts first (tiny, land immediately), staged through a DVE copy:
        # consumers then depend on DVE program order instead of a DMA
        # semaphore (walrus 1-wait limit)
        # xT arrives t-chunk-major ([4, C, 512] in DRAM, chunk-major SBUF
        # tile): fully-contiguous 4KB-per-partition descriptor lines. The
        # first chunk (tb=0) is split three ways across all hwdge queues so
        # the first projection group can start ~13us in; later chunks
        # alternate sync/scalar halves.
        nc.sync.dma_start(wq_s[:, :, 0:128], wq_r[0])
        nc.scalar.dma_start(xT_s[:, 0, 0:3, :], xT_r[0, :, 0:3, :])
        nc.gpsimd.dma_start(xT_s[:, 0, 3:6, :], xT_r[0, :, 3:6, :])
        nc.sync.dma_start(xT_s[:, 0, 6:CT, :], xT_r[0, :, 6:CT, :])
        nc.gpsimd.dma_start(wk_s[:, :, 0:128], wk_r[0])
        nc.gpsimd.dma_start(mask_st[:], mask_d.ap())
        nc.gpsimd.dma_start(bq_st[:], bq_d.ap().rearrange("(o p) -> p o", p=128))
        nc.gpsimd.dma_start(bk_st[:], bk_d.ap().rearrange("(o p) -> p o", p=128))
        for _tb in range(1, IB):
            nc.sync.dma_start(xT_s[:, _tb, 0:4, :], xT_r[_tb, :, 0:4, :])
            nc.scalar.dma_start(xT_s[:, _tb, 4:CT, :], xT_r[_tb, :, 4:CT, :])
            if _tb == 1:
                nc.gpsimd.dma_start(wv_s[:], wv_r[:])
        nc.gpsimd.dma_start(wq_s[:, :, 128:MH], wq_r[1])
        nc.gpsimd.dma_start(wk_s[:, :, 128:MH], wk_r[1])
        wp_r = wp_d.ap().rearrange("(o p) n -> p o n", p=128)
        nc.gpsimd.dma_start(wp_s[:], wp_r[:])
        nc.vector.tensor_copy(mask_s[:], mask_st[:])
        nc.vector.tensor_copy(bq_s[:], bq_st[:])
        nc.vector.tensor_copy(bk_s[:], bk_st[:])
        nc.vector.memset(v_s[:, :, :, D : D + 1], 1.0)

        def proj_group(ps, w_s, b_s, dst, mt, tb, on_scalar=True):
            """one [128, 512] column block of qT or kT (8-deep K accum)."""
            for ct in range(CT):
                nc.tensor.matmul(
                    ps[:, 0:512],
                    w_s[:, ct, mt * 128 : (mt + 1) * 128],
                    xT_s[:, tb, ct, :],
                    start=(ct == 0),
                    stop=(ct == CT - 1),
                )
            if on_scalar:
                nc.scalar.activation(
                    dst[:, mt, tb * 512 : (tb + 1) * 512],
                    ps[:, 0:512],
                    mybir.ActivationFunctionType.Identity,
                    bias=b_s[:, mt : mt + 1],
                )
            else:
                nc.vector.tensor_scalar(
                    dst[:, mt, tb * 512 : (tb + 1) * 512],
                    ps[:, 0:512],
                    b_s[:, mt : mt + 1],
                    None,
                    mybir.AluOpType.add,
                )

        def v_group(tt, ps, on_scalar=True):
            # v natural [t, m]  (xT stationary)
            for ct in range(CT):
                nc.tensor.matmul(
                    ps[:, 0:MH],
                    xT_s[:, tt // 4, ct, (tt % 4) * 128 : (tt % 4) * 128 + 128],
                    wv_s[:, ct, :],
                    start=(ct == 0),
                    stop=(ct == CT - 1),
                )
            if on_scalar:
                nc.scalar.activation(
                    v_s[:, tt, :, 0:D],
                    ps[:, 0:MH].rearrange("p (h d) -> p h d", h=LH),
                    mybir.ActivationFunctionType.Copy,
                )
            else:
                nc.vector.tensor_copy(
                    v_s[:, tt, :, 0:D],
                    ps[:, 0:MH].rearrange("p (h d) -> p h d", h=LH),
                )

        # ---- attention with early-start exp -----------------------------
        # Scalar (exp) is the saturated engine through the middle of the
        # run, so the first score chunks + exps are emitted inside the
        # projection tail: the 83us exp stream starts ~17us earlier.
        # PSUM: att(4) + yt(2) + proj(2) banks during pair 0; the proj
        # pool swaps out for the out-proj pool at pair 1.
        with (
            tc.tile_pool(name="pt", bufs=1) as ptp,
            tc.tile_pool(name="att_ps", bufs=2, space="PSUM") as aps,
            tc.tile_pool(name="norm", bufs=2) as npool,
            tc.tile_pool(name="out_sb", bufs=4) as osb,
            tc.tile_pool(name="yt_ps", bufs=2, space="PSUM") as yps,
        ):
            out_r = out_d.ap().rearrange("(tt p) n -> tt p n", p=128)
            yTu_by_p = [None, None]
            pps_cm = tc.tile_pool(name="proj_ps", bufs=2, space="PSUM")
            pps = pps_cm.__enter__()
            fill_pool = [(pps, "proj_ps")]
            ops_box = [None]

            def v_group(tt, ps, on_scalar=False):
                # v natural [t, m]  (xT stationary)
                for ct in range(CT):
                    nc.tensor.matmul(
                        ps[:, 0:MH],
                        xT_s[:, tt // 4, ct, (tt % 4) * 128 : (tt % 4) * 128 + 128],
                        wv_s[:, ct, :],
                        start=(ct == 0),
                        stop=(ct == CT - 1),
                    )
                if on_scalar:
                    nc.scalar.activation(
                        v_s[:, tt, :, 0:D],
                        ps[:, 0:MH].rearrange("p (h d) -> p h d", h=LH),
                        mybir.ActivationFunctionType.Copy,
                    )
                else:
                    nc.vector.tensor_copy(
                        v_s[:, tt, :, 0:D],
                        ps[:, 0:MH].rearrange("p (h d) -> p h d", h=LH),
                    )

            def attT_chunk(p, PT, jt, off):
                """attT + exp (+ diag mask) for row jt, cols off..off+1024."""
                ia = 128 * jt
                base = TRI_OFF[jt]
                cw = min(1024, T - ia - off)
                for lh in range(2):
                    att_ps = aps.tile([128, 1024], F32, tag="att_ps")
                    prow = slice(64 * lh, 64 * lh + 64)
                    for s5 in range(0, cw, 512):
                        nn = min(512, cw - s5)
                        nc.tensor.matmul(
                            att_ps[:, s5 : s5 + nn],
                            kT_s[prow, p, jt * 128 : (jt + 1) * 128],
                            qT_s[prow, p, ia + off + s5 : ia + off + s5 + nn],
                            start=True,
                            stop=True,
                        )
                    nc.scalar.activation(
                        PT[lh][:, base + off : base + off + cw],
                        att_ps[:, :cw],
                        mybir.ActivationFunctionType.Exp,
                        scale=float(SCALE),
                    )
                    if off == 0:
                        # diagonal 128x128 tile: zero j > i
                        nc.vector.tensor_tensor(
                            PT[lh][:, base : base + 128],
                            PT[lh][:, base : base + 128],
                            mask_s[:],
                            mybir.AluOpType.mult,
                        )

            def attT_rows(p, PT, jts, off0=0):
                """chunk-major over a group of rows: all chunk0 first."""
                offs = [off0] * len(jts)
                while True:
                    emitted_any = False
                    for k, jt in enumerate(jts):
                        if offs[k] < T - 128 * jt:
                            attT_chunk(p, PT, jt, offs[k])
                            offs[k] += 1024
                            emitted_any = True
                    if not emitted_any:
                        break

            def new_PT():
                return [
                    ptp.tile([128, TRI], BF16, tag=f"PT{lh}", name=f"PT{lh}")
                    for lh in range(2)
                ]

            # ---- phase 1: mt=0 projections + v, pair-0 scores start -----
            PT0 = new_PT()
            _mark(nc, "proj_qkv")
            for tb in range(IB):
                late = tb >= 2   # scalar is exp-busy from here on
                for w_s, b_s, dst in ((wq_s, bq_s, qT_s), (wk_s, bk_s, kT_s)):
                    ps = pps.tile([128, 512], F32, tag="proj_ps", name="proj_ps")
                    proj_group(ps, w_s, b_s, dst, 0, tb, on_scalar=not late)
                if tb == 1:
                    for tt in (0, 1):
                        ps = pps.tile([128, 512], F32, tag="proj_ps", name="v_ps")
                        v_group(tt, ps, on_scalar=True)
                elif tb == 2:
                    attT_chunk(0, PT0, 0, 0)
                    attT_chunk(0, PT0, 1, 0)
                    for tt in (2, 3):
                        ps = pps.tile([128, 512], F32, tag="proj_ps", name="v_ps")
                        v_group(tt, ps)
                elif tb == 3:
                    attT_chunk(0, PT0, 2, 0)
                    attT_chunk(0, PT0, 3, 0)
                    for tt in (4, 5, 6, 7):
                        ps = pps.tile([128, 512], F32, tag="proj_ps", name="v_ps")
                        v_group(tt, ps)

            # PE filler work for the scalar-bound attention stretches:
            # mt=1 q/k projection groups and the deferred v tiles 8-15
            def _mt1(w_s, b_s, dst, tb):
                def emit():
                    pool, tag = fill_pool[0]
                    ps = pool.tile([128, 512], F32, tag=tag, name="fill_ps")
                    proj_group(ps, w_s, b_s, dst, 1, tb, on_scalar=False)
                return emit

            def _vg(tt):
                def emit():
                    pool, tag = fill_pool[0]
                    ps = pool.tile([128, 512], F32, tag=tag, name="fill_ps")
                    v_group(tt, ps)
                return emit

            fill_sched = {
                (0, 0): [_mt1(wq_s, bq_s, qT_s, 0), _mt1(wq_s, bq_s, qT_s, 1),
                         _vg(8), _vg(9)],
                (0, 1): [_mt1(wq_s, bq_s, qT_s, 2), _mt1(wq_s, bq_s, qT_s, 3),
                         _vg(10), _vg(11)],
                (0, 2): [_mt1(wk_s, bk_s, kT_s, 0), _vg(12), _vg(13)],
                (0, 3): [_mt1(wk_s, bk_s, kT_s, 1), _vg(14), _vg(15)],
                (1, -1): [_mt1(wk_s, bk_s, kT_s, 2)],
                (1, 0): [_mt1(wk_s, bk_s, kT_s, 3)],
            }

            def fill(key):
                for emit in fill_sched.pop(key, []):
                    emit()

            for p in range(2):
                PT = PT0 if p == 0 else new_PT()
                yTu = npool.tile([64, 8, 512], BF16, tag="yTu", name="yTu")
                yTu_by_p[p] = yTu

                def av_block(ib):
                    """attention @ v for i-block ib; returns yT_ps pair."""
                    yT_ps = [
                        yps.tile([D + 1, 512], F32, tag="yT_ps", name=f"yT_ps{lh}")
                        for lh in range(2)
                    ]
                    for jt in range(4 * ib + 4):
                        for lh in range(2):
                            ia = 128 * jt
                            c0 = max(512 * ib, ia)
                            nc.tensor.matmul(
                                yT_ps[lh][:, c0 - 512 * ib : 512],
                                v_s[:, jt, 2 * p + lh, :],
                                PT[lh][
                                    :,
                                    TRI_OFF[jt]
                                    + c0
                                    - ia : TRI_OFF[jt]
                                    + 512 * ib
                                    + 512
                                    - ia,
                                ],
                                start=(jt == 0),
                                stop=(jt == 4 * ib + 3),
                            )
                    return yT_ps

                def stash_recip(ib, yT_ps):
                    """denominators + 1/s first (they gate s_mults), then y."""
                    st = npool.tile([1, 1024], F32, tag="st", name="st", bufs=1)
                    for lh in range(2):
                        nc.vector.tensor_copy(
                            st[0:1, lh * 512 : (lh + 1) * 512], yT_ps[lh][D : D + 1, :]
                        )
                    rf = npool.tile([1, 1024], F32, tag="rf", name="rf", bufs=1)
                    nc.vector.reciprocal_approx_fast(rf[:], st[:])
                    with nc.allow_low_precision(
                        reason="1/s broadcast in bf16; ~0.4% noise ok"
                    ):
                        nc.vector.tensor_copy(
                            rs_all[0:1, p, ib, :, :].rearrange("a l c -> a (l c)"),
                            rf[:],
                        )
                    for lh in range(2):
                        nc.vector.tensor_copy(yTu[:, ib * 2 + lh, :], yT_ps[lh][0:D, :])

                def sm_pair(ib, pp):
                    """broadcast 1/s (gpsimd partition bcast), scale into yTn."""
                    for lh in range(2):
                        Sb = npool.tile([64, 512], BF16, tag="Sb", name="Sb")
                        nc.gpsimd.partition_broadcast(
                            Sb[:], rs_all[0:1, pp, ib, lh, :]
                        )
                        nc.vector.tensor_tensor(
                            yTn_s[64 * lh : 64 * lh + 64, pp, 512 * ib : 512 * ib + 512],
                            yTu_by_p[pp][:, ib * 2 + lh, :],
                            Sb[:],
                            mybir.AluOpType.mult,
                        )

                def outproj(ib):
                    ops = ops_box[0]
                    # finer DMA units spread over whichever queues are idle
                    # at that stage; 3-way fan-out for the final block so
                    # the trailing drain after the last cast is one hop
                    queues = [
                        [nc.sync, nc.gpsimd],
                        [nc.sync, nc.gpsimd],
                        [nc.sync, nc.scalar],
                        [nc.sync, nc.scalar, nc.gpsimd],
                    ][ib]
                    for tt in range(4 * ib, 4 * ib + 4):
                        ot = osb.tile([128, 1024], BF16, tag="out_t")
                        for nb in range(2):
                            o_ps = ops.tile([128, 512], F32, tag="out_ps", name="o_ps")
                            for pp in range(2):
                                nc.tensor.matmul(
                                    o_ps[:],
                                    yTn_s[:, pp, tt * 128 : (tt + 1) * 128],
                                    wp_s[:, pp, nb * 512 : (nb + 1) * 512],
                                    start=(pp == 0),
                                    stop=(pp == 1),
                                )
                            with nc.allow_low_precision(
                                reason="bf16 output partials; host sums in f32"
                            ):
                                nc.vector.tensor_copy(
                                    ot[:, nb * 512 : (nb + 1) * 512], o_ps[:]
                                )
                            q = queues[(tt * 2 + nb) % len(queues)]
                            q.dma_start(
                                out_r[tt, :, nb * 512 : (nb + 1) * 512],
                                ot[:, nb * 512 : (nb + 1) * 512],
                            )

                if p == 0:
                    for ib in range(IB):
                        _mark(nc, f"p0_att{ib}")
                        # rows 0-3 already emitted their first chunks in
                        # the projection tail
                        attT_rows(p, PT, range(4 * ib, 4 * ib + 4),
                                  off0=1024 if ib == 0 else 0)
                        _mark(nc, f"p0_fill{ib}")
                        fill((0, ib))
                        _mark(nc, f"p0_av{ib}")
                        yT_ps = av_block(ib)
                        stash_recip(ib, yT_ps)
                    # proj psum pool swaps out for the out-proj pool
                    pps_cm.__exit__(None, None, None)
                    ops_cm = tc.tile_pool(name="out_ps", bufs=2, space="PSUM")
                    ops_box[0] = ops_cm.__enter__()
                    fill_pool[0] = (ops_box[0], "out_ps")
                else:
                    # prologue: two i-blocks of attT ahead, first av staged;
                    # the late kT mt=1 groups land here (needed only from
                    # attT row 8 onward). Pair-0's scale-mults all drain
                    # here instead of serializing into the tail.
                    _mark(nc, "p1_att0")
                    attT_rows(p, PT, range(0, 4))
                    fill((1, -1))
                    for _ib in range(IB):
                        sm_pair(_ib, 0)
                    _mark(nc, "p1_av0")
                    yT_ps = av_block(0)
                    stash_recip(0, yT_ps)
                    _mark(nc, "p1_att1")
                    attT_rows(p, PT, range(4, 8))
                    for ib in range(IB):
                        _mark(nc, f"p1_sm{ib}")
                        sm_pair(ib, 1)
                        if ib + 1 < IB:
                            _mark(nc, f"p1_av{ib + 1}")
                            yT_ps = av_block(ib + 1)
                            stash_recip(ib + 1, yT_ps)
                        if ib == 0:
                            fill((1, 0))
                        if ib + 2 < IB:
                            _mark(nc, f"p1_att{ib + 2}")
                            attT_rows(p, PT, range(4 * (ib + 2), 4 * (ib + 2) + 4))
                        _mark(nc, f"p1_out{ib}")
                        outproj(ib)
                    ops_cm.__exit__(None, None, None)



_NC_CACHE = None


def get_nc() -> bass.Bass:
    global _NC_CACHE
    if _NC_CACHE is None:
        nc = bacc.Bacc()
        xT_d = nc.declare_dram_parameter("xT", [IB, C, 512], BF16, isOutput=False)
        wq_d = nc.declare_dram_parameter("wq", [2, C, 128], BF16, isOutput=False)
        wk_d = nc.declare_dram_parameter("wk", [2, C, 128], BF16, isOutput=False)
        wv_d = nc.declare_dram_parameter("wv", [C, MH], BF16, isOutput=False)
        wp_d = nc.declare_dram_parameter("wp", [MH, C], BF16, isOutput=False)
        bq_d = nc.declare_dram_parameter("bq", [MH], F32, isOutput=False)
        bk_d = nc.declare_dram_parameter("bk", [MH], F32, isOutput=False)
        out_d = nc.declare_dram_parameter("out", [T, C], BF16, isOutput=True)
        mask_d = nc.inline_tensor(_causal_mask(), name="causal_mask")
        emit_kernel(
            nc, xT_d, wq_d, wk_d, wv_d, wp_d, bq_d, bk_d, out_d, mask_d
        )
        nc.finalize()
        _NC_CACHE = nc
    return _NC_CACHE


def make_in_maps(x, Wq, bq, Wk, bk, Wv, bv, Wp, bp):
    in_maps = []
    for core in range(N_CORES):
        b, g = divmod(core, GROUPS)
        sl = slice(g * MH, (g + 1) * MH)
        in_maps.append(
            {
                "xT": np.ascontiguousarray(
                    x[b].T.reshape(C, 4, 512).transpose(1, 0, 2)
                ).astype(NP_BF16),
                "wq": np.ascontiguousarray(
                    np.stack([Wq[:, sl][:, 0:128], Wq[:, sl][:, 128:MH]])
                ).astype(NP_BF16),
                "wk": np.ascontiguousarray(
                    np.stack([Wk[:, sl][:, 0:128], Wk[:, sl][:, 128:MH]])
                ).astype(NP_BF16),
                "wv": np.ascontiguousarray(Wv[:, sl]).astype(NP_BF16),
                "wp": np.ascontiguousarray(Wp[sl, :]).astype(NP_BF16),
                "bq": np.ascontiguousarray(bq[sl]).astype(np.float32),
                "bk": np.ascontiguousarray(bk[sl]).astype(np.float32),
            }
        )
    return in_maps


def kernel(x, Wq, bq, Wk, bk, Wv, bv, Wp, bp, _results_hook=None, _trace=False):
    x = np.asarray(x, dtype=np.float32)
    nc = get_nc()
    in_maps = make_in_maps(x, Wq, bq, Wk, bk, Wv, bv, Wp, bp)
    res = run_bass_kernel_spmd(
        nc, in_maps, core_ids=list(range(N_CORES)), trace=_trace
    )
    if _results_hook is not None:
        _results_hook(res)
    out = np.zeros((B, T, C), dtype=np.float32)
    for core in range(N_CORES):
        b = core // GROUPS
        out[b] += np.asarray(res.results[core]["out"], dtype=np.float32)
    # v-bias folds through softmax exactly (attention rows sum to 1):
    # y = att @ (v + 1 bv^T)  =>  out += bv @ Wp, plus the output bias bp
    bias_row = (
        np.asarray(bv, dtype=np.float32) @ np.asarray(Wp, dtype=np.float32)
        + np.asarray(bp, dtype=np.float32)
    )
    out += bias_row[None, None, :]
    return out

